# revision 28
# baseline (speedup 1.0000x reference)
"""Trainium2 Bass kernel for a binarized (1w1a) BasicBlock:

    out = BN2(PReLU(conv3x3(sign(x1), std2*sign(W2)) + b2)) + x1
    x1  = BN1(PReLU(conv3x3(sign(x),  std1*sign(W1)) + b1)) + x

Strategy
--------
Data-parallel over the batch axis: each of the 8 NeuronCores processes 8 of
the 64 images, with the (small) weights / BN / PReLU params replicated.
No collectives are needed.

Per-core compute:
  * Activations and weights are binarized (sign -> {-1, 0, +1}), which is
    exactly representable in fp8e4m3.  The 3x3 conv over 256->256 channels is
    expressed as 9 shifted matmuls accumulating in PSUM, using the fp8
    DoubleRow perf mode so each matmul contracts the full K=256 input
    channels (2 fp8 weights per PE cell).
  * Images are stored in a zero-padded "strip" layout: per partition, each
    image is a 33x33 plane (32 real rows + 1 pad row, 32 real cols + 1 pad
    col).  8 planes are concatenated into one 8712-element strip per
    channel-half, with zero guard regions at both ends.  Every 3x3 tap is
    then a constant offset into the strip, so a whole 15-row window
    (495 outputs <= 1 PSUM bank) is computed with uniform-stride APs.
  * PSUM evacuation folds conv-bias + BN into one ScalarE activation with
    per-partition scale/bias:  t = (g*std)*S + (g*b + beta - mean*g).
    PReLU (0<alpha<1) is one fused VectorE op: max(alpha*t, t), and the
    residual + post-BN shift is a second fused op: (p + d) + x.
  * sign(out1) is written interior-only (skipping pad rows/cols) into a
    pre-zeroed fp8 strip, which feeds conv2's matmuls.

The host side only reshapes/transposes/zero-pads (layout), shards the batch
and un-packs the output strip.  All arithmetic (sign, BN folding, conv,
PReLU, residual) happens on-device.
"""

import math
import os
import sys

import numpy as np

for _p in ("/opt/trn_rl_repo", "/root/.axon_site/_ro/trn_rl_repo"):
    if os.path.isdir(_p) and _p not in sys.path:
        sys.path.insert(0, _p)

import concourse.bass as bass
import concourse.bacc as bacc
import concourse.mybir as mybir
from concourse import tile
from concourse.bass_utils import run_bass_kernel_spmd

F32 = mybir.dt.float32
F8 = mybir.dt.float8e4
AOP = mybir.AluOpType
AFT = mybir.ActivationFunctionType
DR = mybir.MatmulPerfMode.DoubleRow

EPS = 1e-5
NCORES = 8
NIMG = 8            # images per core
NROW = 33           # rows per image plane (32 real + 1 pad)
NCOL = 33           # cols per plane (32 real + 1 pad)
PLANE = NROW * NCOL             # 1089
STRIP = NIMG * PLANE            # 8712
TOTROWS = NIMG * NROW           # 264
GF = 64                         # front guard (>= 34)
GB = 72                         # back guard; GF+STRIP+GB divisible by 16
SLEN = GF + STRIP + GB          # 8848
WROWS = 15                      # strip rows per window
STD = math.sqrt(2.0) / math.sqrt(256 * 9)

# window table: (row0, nrows, elem0, nelem)
WINDOWS = []
_r = 0
while _r < TOTROWS:
    nr = min(WROWS, TOTROWS - _r)
    WINDOWS.append((_r, nr, _r * NCOL, nr * NCOL))
    _r += nr

# split of each fp8 strip into two tiles at window SPLITW (row 150) so the
# dependency chains of early and late windows are decoupled (Tile tracks
# deps per tile).  Rows 149/150 live in both tiles (3x3 taps reach +-1 row).
SPLITW = 10
SPLITROW = SPLITW * WROWS            # 150
B_BASE_ROW = SPLITROW - 1            # 149
B_BASE_E = B_BASE_ROW * NCOL         # 4917
ALEN = (SPLITROW + 1) * NCOL         # 4983 (rows 0..150)
BLEN = TOTROWS * NCOL - B_BASE_E     # 3795 (rows 149..263)
SLEN_A = GF + ALEN + 9               # 5056, 16-aligned
SLEN_B = GF + BLEN + 45              # 3904, 16-aligned


def route_rows(ra, rb):
    """Map absolute strip-row range [ra, rb) onto the two strip tiles."""
    out = []
    lo, hi = max(ra, 0), min(rb, SPLITROW + 1)
    if lo < hi:
        out.append((0, lo, hi))          # tile A, base row 0
    lo, hi = max(ra, B_BASE_ROW), min(rb, TOTROWS)
    if lo < hi:
        out.append((1, lo, hi))          # tile B, base row B_BASE_ROW
    return out


# pair table: consecutive window pairs for batched post-ops
PAIRS = []
_i = 0
while _i < len(WINDOWS):
    take = 2 if _i + 2 < len(WINDOWS) else 1   # last two windows stay single
    ws_ = WINDOWS[_i:_i + take]
    r0 = ws_[0][0]
    nr = sum(w[1] for w in ws_)
    e0 = ws_[0][2]
    ln = sum(w[3] for w in ws_)
    PAIRS.append((list(range(_i, _i + len(ws_))), r0, nr, e0, ln))
    _i += take
PAIR_IDX = {p[0][0]: k for k, p in enumerate(PAIRS)}

# per-channel param column order inside the packed [128, 22] table
PARAM_ORDER = [
    "b1", "alpha", "bn1_gamma", "bn1_beta", "bn1_mean", "bn1_var",
    "b2", "bn2_gamma", "bn2_beta", "bn2_mean", "bn2_var",
]
NPARAM = len(PARAM_ORDER)


def _real_runs(r0, nr):
    """Runs of consecutive non-pad strip rows within [r0, r0+nr)."""
    runs = []
    a = None
    for r in range(r0, r0 + nr):
        if r % NROW == NROW - 1:  # pad row
            if a is not None:
                runs.append((a, r))
                a = None
        else:
            if a is None:
                a = r
    if a is not None:
        runs.append((a, r0 + nr))
    return runs


def _rows_ap2(t3, a, b, base=0):
    """[128, 2, b-a, 32] AP over real cols of strip rows [a,b), both halves."""
    ap = t3[:, :, base + a * NCOL: base + b * NCOL]
    ap = ap.rearrange("p i (r c) -> p i r c", c=NCOL)
    return ap[:, :, :, :32]


def _rows_ap(t3, m, a, b, base=0):
    """[128, b-a, 32] AP over real columns of strip rows [a, b) of t3[:, m]."""
    ap = t3[:, m, base + a * NCOL: base + b * NCOL]
    ap = ap.rearrange("p (r c) -> p r c", c=NCOL)
    return ap[:, :, :32]


def build_program():
    nc = bacc.Bacc("TRN2", target_bir_lowering=False, debug=False,
                   num_devices=NCORES)

    xs = nc.declare_dram_parameter("xs", [2, 128, STRIP], F32, isOutput=False)
    w1 = nc.declare_dram_parameter("w1", [128, 18, 2, 128], F32, isOutput=False)
    w2 = nc.declare_dram_parameter("w2", [128, 18, 2, 128], F32, isOutput=False)
    pv = nc.declare_dram_parameter("pv", [128, 2 * NPARAM], F32, isOutput=False)
    outd = nc.declare_dram_parameter("out", [2, 128, STRIP], F32, isOutput=True)

    with tile.TileContext(nc) as tc:
        with (
            tc.tile_pool(name="big", bufs=1) as big,
            tc.tile_pool(name="wstage", bufs=1) as wsp,
            tc.tile_pool(name="xw", bufs=6) as xwp,
            tc.tile_pool(name="t2", bufs=4) as t2p,
            tc.tile_pool(name="psum", bufs=8, space="PSUM") as psp,
        ):
            s1a = big.tile([128, 2, SLEN_A], F8, tag="s1a")
            s1b = big.tile([128, 2, SLEN_B], F8, tag="s1b")
            s2a = big.tile([128, 2, SLEN_A], F8, tag="s2a")
            s2b = big.tile([128, 2, SLEN_B], F8, tag="s2b")
            o1 = big.tile([128, 2, STRIP], F32, tag="o1")
            w1f = big.tile([128, 18, 2, 128], F8, tag="w1f")
            w2f = big.tile([128, 18, 2, 128], F8, tag="w2f")
            pt = big.tile([128, 2 * NPARAM], F32, tag="pt")
            dv = big.tile([128, 12], F32, tag="dv")  # per conv: sc, g*b, d
            scr = big.tile([128, 8], F32, tag="scr")
            scr2 = big.tile([128, 8], F32, tag="scr2")

            # ---- params first: everything derived hangs off this DMA --
            nc.gpsimd.dma_start(out=pt[:, :], in_=pv[:, :])
            # dummy ACTIVATE with no data deps: forces the ACT table load
            # to happen immediately instead of before the first real sign
            nc.scalar.sign(out=scr2[:, 6:7],
               in_=nc.const_aps.tensor(0.0, (128, 1)))

            # s1's interior is fully overwritten by sign1 (pads included,
            # since the host-baked x strip has zero pads), so only its guard
            # regions need zeroing.  s2's pads are never written (sign2 is
            # interior-only), so it gets a full clear via a u32 view.
            U32 = mybir.dt.uint32
            nc.vector.memset(s1a[:, :, 0:GF].bitcast(U32), 0)
            nc.vector.memset(s1a[:, :, GF + ALEN:SLEN_A], 0.0)
            nc.vector.memset(s1b[:, :, 0:GF].bitcast(U32), 0)
            nc.vector.memset(s1b[:, :, GF + BLEN:SLEN_B], 0.0)

            # conv1 weights immediately (gates the first matmuls): DMA via
            # GpSimd queue, binarize on VectorE to (w>0)-0.5 = +-0.5
            for h in range(2):
                ws = wsp.tile([128, 9, 2, 128], F32, tag="ws", name="ws")
                nc.gpsimd.dma_start(out=ws[:, :, :, :],
                                    in_=w1[:, h * 9:(h + 1) * 9, :, :])
                nc.vector.tensor_scalar(w1f[:, h * 9:(h + 1) * 9, :, :],
                                        ws[:, :, :, :], 0.0, 0.5,
                                        AOP.is_gt, AOP.subtract)

            # ---- x pair-windows: DMA (GpSimd queue) + sign -> s1 ------
            xwt = {}

            def feed_pair(pi):
                if pi in xwt or pi >= len(PAIRS):
                    return
                _wis, r0, nr, e0, ln = PAIRS[pi]
                xb = xwp.tile([128, 2, 2 * WROWS * NCOL], F32, tag="xw",
                              name="xw")
                for i in range(2):
                    nc.gpsimd.dma_start(out=xb[:, i, :ln],
                                        in_=xs[i, :, e0:e0 + ln])
                for t, lo, hi in route_rows(r0, r0 + nr):
                    dst = (s1a, 0) if t == 0 else (s1b, B_BASE_ROW)
                    o0 = GF + (lo - dst[1]) * NCOL
                    nc.scalar.sign(
                        out=dst[0][:, :, o0: o0 + (hi - lo) * NCOL],
                        in_=xb[:, :, (lo - r0) * NCOL:(hi - r0) * NCOL])
                xwt[pi] = xb

            for pi in range(4):
                feed_pair(pi)


            def pcol(m, name):
                k = PARAM_ORDER.index(name)
                return pt[:, m * NPARAM + k: m * NPARAM + k + 1]

            def dcol(j):
                return dv[:, j: j + 1]

            def scol(j):
                return scr[:, j: j + 1]

            # Batched rsqrt(var+eps) for all 4 (conv, half) columns at
            # once: Quake-III bit-trick seed + 3 Newton iterations, all on
            # VectorE (no ScalarE Sqrt -> no extra ACT table load; DVE
            # reciprocal is not needed either).
            vco = [("bn1", 0), ("bn1", 1), ("bn2", 0), ("bn2", 1)]
            vpe = scr[:, 0:4]
            for j, (pfx, m) in enumerate(vco):
                nc.vector.tensor_scalar_add(scr[:, j:j + 1],
                                            pcol(m, pfx + "_var"), EPS)
            yb = scr[:, 4:8]
            nc.vector.memset(yb.bitcast(U32), 0x5f3759df)
            nc.vector.tensor_scalar(scr2[:, 0:4].bitcast(U32),
                                    vpe.bitcast(U32), 1, None,
                                    AOP.logical_shift_right)
            nc.vector.tensor_tensor(yb.bitcast(U32), yb.bitcast(U32),
                                    scr2[:, 0:4].bitcast(U32), AOP.subtract)
            for _ in range(3):
                nc.vector.tensor_tensor(scr2[:, 0:4], yb, yb, AOP.mult)
                nc.vector.tensor_tensor(scr2[:, 0:4], vpe, scr2[:, 0:4],
                                        AOP.mult)
                nc.vector.tensor_scalar(scr2[:, 0:4], scr2[:, 0:4], -0.5, 1.5,
                                        AOP.mult, AOP.add)
                nc.vector.tensor_tensor(yb, yb, scr2[:, 0:4], AOP.mult)

            for j, (pfx, m) in enumerate(vco):
                ci = j // 2
                gam = pcol(m, pfx + "_gamma")
                bet = pcol(m, pfx + "_beta")
                mean = pcol(m, pfx + "_mean")
                bvec = pcol(m, "b1" if ci == 0 else "b2")
                rs = yb[:, j:j + 1]
                g = scr2[:, 4:5]
                nc.vector.tensor_tensor(g, gam, rs, AOP.mult)
                nc.vector.tensor_scalar_mul(dcol(ci * 6 + m), g,
                                            STD * (2.0 if ci == 0 else 1.0))
                nc.vector.tensor_tensor(dcol(ci * 6 + 2 + m), g, bvec, AOP.mult)
                nc.vector.tensor_tensor(scr2[:, 5:6], mean, g, AOP.mult)
                nc.vector.tensor_tensor(dcol(ci * 6 + 4 + m), bet,
                                        scr2[:, 5:6], AOP.subtract)

            def sc_ap(conv, m):
                return dcol((conv - 1) * 6 + m)

            def bi_ap(conv, m):
                return dcol((conv - 1) * 6 + 2 + m)

            def dd_ap(conv, m):
                return dcol((conv - 1) * 6 + 4 + m)

            def al_ap(m):
                return pcol(m, "alpha")

            # s2 cleared after the startup-critical work is queued
            for i in range(2):
                nc.vector.memset(s2a[:, i, :].bitcast(U32), 0)
                nc.vector.memset(s2b[:, i, :].bitcast(U32), 0)
            # conv2 weights: ScalarE Sign -> {-1, +1} (not startup-critical)
            for h in range(2):
                ws = wsp.tile([128, 9, 2, 128], F32, tag="ws", name="ws")
                nc.gpsimd.dma_start(out=ws[:, :, :, :],
                                    in_=w2[:, h * 9:(h + 1) * 9, :, :])
                nc.scalar.sign(out=w2f[:, h * 9:(h + 1) * 9, :, :],
                               in_=ws[:, :, :, :])

            # ---- the two convs ---------------------------------------
            def conv_group(srcab, wf8, convno, pgrp, tail_split=False):
                first = convno == 1
                if True:
                    for m in range(2):
                        ps = {}
                        for pr in pgrp:
                            for wi in pr[0]:
                                ps[wi] = psp.tile([128, WROWS * NCOL], F32,
                                                  tag="ps", name="ps")
                        for tap in range(9):
                            dy, dx = divmod(tap, 3)
                            off = (dy - 1) * NCOL + (dx - 1)
                            lhsT = wf8[:, tap * 2 + m, :, :]
                            for pr in pgrp:
                                for wi in pr[0]:
                                    r0, nr, e0, ln = WINDOWS[wi]
                                    if wi < SPLITW:
                                        st, c0 = srcab[0], GF + e0 + off
                                    else:
                                        st = srcab[1]
                                        c0 = GF + (e0 - B_BASE_E) + off
                                    nc.tensor.matmul(
                                        ps[wi][:, :ln], lhsT,
                                        st[:, :, c0: c0 + ln],
                                        start=(tap == 0), stop=(tap == 8),
                                        perf_mode=DR)
                        for pr in pgrp:
                            wis, r0, nr, e0, ln = pr
                            pi = PAIR_IDX[wis[0]]
                            if first:
                                dst = o1[:, m, e0:e0 + ln]
                            else:
                                tb = t2p.tile([128, 2 * WROWS * NCOL], F32,
                                              tag="t2", name="t2")
                                dst = tb[:, :ln]
                            # evacuate each window's psum into the pair buffer
                            o_off = 0
                            for wi in wis:
                                _r0, _nr, _e0, _ln = WINDOWS[wi]
                                nc.scalar.activation(
                                    dst[:, o_off:o_off + _ln] if not first
                                    else o1[:, m, _e0:_e0 + _ln],
                                    ps[wi][:, :_ln], AFT.Identity,
                                    bias=bi_ap(convno, m),
                                    scale=sc_ap(convno, m))
                                o_off += _ln
                            # PReLU + (residual + d) at pair granularity
                            nc.vector.scalar_tensor_tensor(
                                dst, dst, al_ap(m), dst, AOP.mult, AOP.max)
                            if first:
                                nc.vector.scalar_tensor_tensor(
                                    dst, dst, dd_ap(convno, m),
                                    xwt[pi][:, m, :ln], AOP.add, AOP.add)
                                # sign(out1) interior-only -> s2
                                for a, b in _real_runs(r0, nr):
                                    for t, lo, hi in route_rows(a, b):
                                        dt_ = (s2a, 0) if t == 0 \
                                            else (s2b, B_BASE_ROW)
                                        nc.scalar.sign(
                                            out=_rows_ap(dt_[0], m,
                                                         lo - dt_[1],
                                                         hi - dt_[1], base=GF),
                                            in_=_rows_ap(o1, m, lo, hi))
                            elif tail_split:
                                nc.scalar.activation(dst, dst, AFT.Identity,
                                                     bias=dd_ap(convno, m),
                                                     scale=1.0)
                                nc.gpsimd.tensor_tensor(
                                    dst, dst, o1[:, m, e0:e0 + ln], AOP.add)
                                nc.sync.dma_start(
                                    out=outd[m, :, e0:e0 + ln], in_=dst)
                            else:
                                nc.vector.scalar_tensor_tensor(
                                    dst, dst, dd_ap(convno, m),
                                    o1[:, m, e0:e0 + ln], AOP.add, AOP.add)
                                nc.sync.dma_start(
                                    out=outd[m, :, e0:e0 + ln], in_=dst)

            pgroups = [PAIRS[g:g + 2] for g in range(0, len(PAIRS), 2)]
            for gi, pg in enumerate(pgroups):
                conv_group((s1a, s1b), w1f, 1, pg)
                feed_pair(2 * gi + 4)
                feed_pair(2 * gi + 5)
            for pg in pgroups:
                conv_group((s2a, s2b), w2f, 2, pg)

    nc.compile()
    return nc


# ---------------------------------------------------------------- host side

def _host_pack_x(x_shard):
    """[8,256,32,32] f32 -> strip layout [2,128,STRIP] with zero pads."""
    xz = np.zeros((2, 128, NIMG, NROW, NCOL), dtype=np.float32)
    xr = x_shard.reshape(NIMG, 2, 128, 32, 32)
    xz[:, :, :, :32, :32] = xr.transpose(1, 2, 0, 3, 4)
    return np.ascontiguousarray(xz.reshape(2, 128, STRIP))


def _host_pack_w(W):
    """[256,256,3,3] -> [128(k), 18(tap*2+m), 2(i), 128(j)] f32."""
    A = np.asarray(W, dtype=np.float32).reshape(2, 128, 2, 128, 3, 3)
    L = A.transpose(3, 4, 5, 0, 2, 1)          # (k, dy, dx, m, i, j)
    L = L.reshape(128, 9, 2, 2, 128)           # (k, tap, m, i, j)
    return np.ascontiguousarray(L.reshape(128, 18, 2, 128))


def _host_pack_pv(inputs):
    pvt = np.zeros((128, 2 * NPARAM), dtype=np.float32)
    for k, name in enumerate(PARAM_ORDER):
        v = np.asarray(inputs[name], dtype=np.float32)
        for m in range(2):
            pvt[:, m * NPARAM + k] = v[m * 128:(m + 1) * 128]
    return pvt


def _host_unpack_out(o):
    """[2,128,STRIP] -> [8,256,32,32]."""
    o = o.reshape(2, 128, NIMG, NROW, NCOL)[:, :, :, :32, :32]
    return np.ascontiguousarray(o.transpose(2, 0, 1, 3, 4)
                                .reshape(NIMG, 256, 32, 32))


_PROG = None
LAST_EXEC_TIME_NS = None


def _get_prog():
    global _PROG
    if _PROG is None:
        _PROG = build_program()
    return _PROG


def kernel(x, W1, b1, W2, b2, alpha,
           bn1_gamma, bn1_beta, bn1_mean, bn1_var,
           bn2_gamma, bn2_beta, bn2_mean, bn2_var,
           _trace=False):
    global LAST_EXEC_TIME_NS
    inputs = dict(b1=b1, b2=b2, alpha=alpha,
                  bn1_gamma=bn1_gamma, bn1_beta=bn1_beta,
                  bn1_mean=bn1_mean, bn1_var=bn1_var,
                  bn2_gamma=bn2_gamma, bn2_beta=bn2_beta,
                  bn2_mean=bn2_mean, bn2_var=bn2_var)
    x = np.asarray(x, dtype=np.float32)
    w1l = _host_pack_w(W1)
    w2l = _host_pack_w(W2)
    pvt = _host_pack_pv(inputs)

    in_maps = []
    for c in range(NCORES):
        shard = x[c * NIMG:(c + 1) * NIMG]
        in_maps.append({"xs": _host_pack_x(shard), "w1": w1l, "w2": w2l,
                        "pv": pvt})

    nc = _get_prog()
    res = run_bass_kernel_spmd(nc, in_maps, core_ids=list(range(NCORES)),
                               trace=_trace)
    LAST_EXEC_TIME_NS = res.exec_time_ns

    outs = [_host_unpack_out(res.results[c]["out"]) for c in range(NCORES)]
    return np.concatenate(outs, axis=0)
